# revision 31
# baseline (speedup 1.0000x reference)
"""AdaptivePConv Trainium2 kernel (8 NeuronCores, data-parallel over batch).

Per core (sample b = core index), on-device:
  1. per-channel min/max (streamed), then exact bins
     q = clip(trunc((x-mn)*recip(mx-mn+eps)*256), 0, 255) stored bf16
     (fp32 magic-number round + is_gt fixup -> exact floor; bf16 holds
      integers 0..255 exactly)
  2. 256-bin histogram entirely on the PE: per 512-col chunk transpose q
     to pixel-major, split into nibbles hi=floor(q/16) (exact via
     RN((q-7.5)/16) since q is integer), lo=q-16*hi; the hi one-hot is
     generated directly in matmul-stationary "slab" layout (col =
     (tile,group)*128 + thi*8 + ch, one strided is_equal per thi), the lo
     one-hot in plane layout read via a 2-dim moving AP; block-diagonal
     [128,128] matmuls accumulate per-stream 16x16 outer-product counts
     (8 (c,h)-streams per matmul) into PSUM
  3. entropy per channel (ACT Ln), top-16 channels via max8/match_replace
  4. selected channels -> 3x3 conv: zero-padded pitched selb, 2 output
     rows per matmul (FD=512 moving, 2-dim AP), bias on ACT
  5. unselected channels scattered to the output tail as 2 big rows per
     channel via indirect DMA (few descriptors), before the conv
"""
import os
import sys
import numpy as np

sys.path.insert(0, "/opt/trn_rl_repo")

import concourse.bass as bass
import concourse.bacc as bacc
import concourse.tile as tile
from concourse import mybir
import concourse.bass_utils as bu

F32 = mybir.dt.float32
BF16 = mybir.dt.bfloat16
FP8 = mybir.dt.float8e4
I32 = mybir.dt.int32
U32 = mybir.dt.uint32
Alu = mybir.AluOpType
Act = mybir.ActivationFunctionType
AxX = mybir.AxisListType.X

B, C, H, W = 8, 64, 256, 256
N = H * W                  # 65536 pixels per channel
NHALF = N // 2             # 32768 per (channel, half) partition row
OC, P_SEL = 64, 16
C_OUT = OC + C - P_SEL     # 112
NBINS = 256
MAGIC = float(np.float32(2.0 ** 23))
MAGIC15 = float(np.float32(1.5 * 2.0 ** 23))  # safe for slightly-negative inputs

RA = 2048                  # leading cols counted via ACT Sign thresholds
PE_COLS = NHALF - RA       # 30720 cols through the PE one-hot route
HCH = 512                  # hist chunk (4 transpose tiles, 2 DR pairs)
TPH = HCH // 128
QCH = 1024                 # q-phase chunk (= 1 hist chunk)
MMCH = 4096                # min/max chunk
PITCH = 258                # padded conv row pitch
SELW3 = PITCH * (H + 1)    # selb3: 257 rows x 258 (row 256 = slack)


def bcast(ap_small, ap_big):
    return bass.broadcast_tensor_aps(ap_small, ap_big)[0]


def build():
    nc = bacc.Bacc()
    x_ext = nc.declare_dram_parameter("x", [C, N], F32, isOutput=False)
    biasA_ext = nc.declare_dram_parameter("biasA", [128, 255], F32, isOutput=False)
    w_ext = nc.declare_dram_parameter("w", [48, 3 * OC], F32, isOutput=False)
    bias_ext = nc.declare_dram_parameter("bias", [OC, 1], F32, isOutput=False)
    ident_ext = nc.declare_dram_parameter("ident", [128, 128], F32, isOutput=False)
    iota_ext = nc.declare_dram_parameter("colio", [1, 64], F32, isOutput=False)
    vpats_ext = nc.declare_dram_parameter("vpats", [128, 512], F32, isOutput=False)
    vpatm_ext = nc.declare_dram_parameter("vpatm", [128, 512], F32, isOutput=False)
    out_ext = nc.declare_dram_parameter("out", [C_OUT, N], F32, isOutput=True)
    dbg_act = nc.declare_dram_parameter("dbg_act", [C, 1], F32, isOutput=True)
    dbg_idx = nc.declare_dram_parameter("dbg_idx", [1, 16], U32, isOutput=True)
    dbg_n = nc.declare_dram_parameter("dbg_n", [C, NBINS], F32, isOutput=True)

    # +W slack: the ky=2 gather's last slice reads 256 elems past channel end
    xbf_d = nc.dram_tensor("xbf", [128 * NHALF + W], BF16)
    scr_stat = nc.dram_tensor("scr_stat", [128], F32)
    scr_stat2 = nc.dram_tensor("scr_stat2", [64], F32)
    scr_cge = nc.dram_tensor("scr_cge", [128, 255], F32)
    scr_act = nc.dram_tensor("scr_act", [64], F32)
    scr_idx = nc.dram_tensor("scr_idx", [16], F32)
    scr_slot = nc.dram_tensor("scr_slot", [64], F32)
    scr_hpe = nc.dram_tensor("scr_hpe", [2 * 64 * 256], F32)

    def dram_ap(t, offset, pattern):
        return bass.AP(t, offset, pattern)

    x_r = x_ext[:].rearrange("c (t m) -> (c t) m", t=2)  # [128, NHALF]

    with tile.TileContext(nc) as tc:
        with tc.tile_pool(name="persist", bufs=1) as pp:
            epsT = pp.tile([128, 1], F32)
            nc.vector.memset(epsT[:], 1e-8)
            zb = pp.tile([128, 1], F32)
            nc.vector.memset(zb[:], 0.0)
            z64 = pp.tile([1, 64], F32)
            nc.vector.memset(z64[:], 0.0)
            colio = pp.tile([1, 64], F32)
            nc.sync.dma_start(colio[:], iota_ext[:])
            vpatS = pp.tile([128, 512], BF16)
            vpatM = pp.tile([128, 512], BF16)
            with tc.tile_pool(name="vptmp", bufs=1) as vpt:
                vpf = vpt.tile([128, 512], F32, tag="vpf")
                nc.sync.dma_start(vpf[:], vpats_ext[:])
                nc.vector.tensor_copy(vpatS[:], vpf[:])
                vpf2 = vpt.tile([128, 512], F32, tag="vpf2")
                nc.sync.dma_start(vpf2[:], vpatm_ext[:])
                nc.vector.tensor_copy(vpatM[:], vpf2[:])
            goffI8 = pp.tile([16, 16], I32)  # ch*16 + j (16th-slice units)
            slotI32 = pp.tile([64, 1], I32)

            with tc.tile_pool(name="qpool", bufs=1) as qp:
                q_sb = qp.tile([128, NHALF], BF16)
                mn128 = qp.tile([128, 1], F32)
                r256 = qp.tile([128, 1], F32)
                identb = qp.tile([128, 128], BF16)
                with tc.tile_pool(name="idtmp", bufs=1) as idt:
                    idf = idt.tile([128, 128], F32)
                    nc.sync.dma_start(idf[:], ident_ext[:])
                    nc.vector.tensor_copy(identb[:], idf[:])
                biasA = qp.tile([128, 255], F32)
                nc.sync.dma_start(biasA[:], biasA_ext[:])
                accA = qp.tile([128, 255], F32)
                junkA = qp.tile([128, RA], BF16)
                b716 = qp.tile([128, 1], F32)
                nc.vector.memset(b716[:], -0.46875)

                # ============ phase A: min/max (streamed) ============
                nmm = NHALF // MMCH
                mxP = qp.tile([128, nmm], F32)
                mnP = qp.tile([128, nmm], F32)
                with tc.tile_pool(name="xmm", bufs=2) as xmp, \
                     nc.named_scope("phA"):
                    for k in range(nmm):
                        sl = slice(k * MMCH, (k + 1) * MMCH)
                        xh = xmp.tile([128, MMCH], F32, tag="xh")
                        nc.sync.dma_start(xh[:], x_r[:, sl])
                        nc.vector.tensor_reduce(mxP[:, k:k + 1], xh[:], axis=AxX, op=Alu.max)
                        nc.vector.tensor_reduce(mnP[:, k:k + 1], xh[:], axis=AxX, op=Alu.min)
                        # bf16 mirror of x for the conv gather (DMA can't cast)
                        xb = xmp.tile([128, MMCH], BF16, tag="xb")
                        nc.scalar.copy(xb[:], xh[:])
                        nc.sync.dma_start(
                            dram_ap(xbf_d, k * MMCH, [[NHALF, 128], [1, MMCH]]),
                            xb[:])
                    mx1 = qp.tile([128, 1], F32)
                    mn1 = qp.tile([128, 1], F32)
                    nc.vector.tensor_reduce(mx1[:], mxP[:], axis=AxX, op=Alu.max)
                    nc.vector.tensor_reduce(mn1[:], mnP[:], axis=AxX, op=Alu.min)

                    # combine (c,half) partition pairs via DRAM bounce
                    mx128 = qp.tile([128, 1], F32)
                    for src1, dst, op in ((mx1, mx128, Alu.max), (mn1, mn128, Alu.min)):
                        nc.sync.dma_start(dram_ap(scr_stat, 0, [[1, 128]]), src1[:])
                        ev = qp.tile([64, 1], F32, tag="ev")
                        od = qp.tile([64, 1], F32, tag="od")
                        nc.sync.dma_start(ev[:], dram_ap(scr_stat, 0, [[2, 64], [1, 1]]))
                        nc.sync.dma_start(od[:], dram_ap(scr_stat, 1, [[2, 64], [1, 1]]))
                        cmb = qp.tile([64, 1], F32, tag="cmb")
                        nc.vector.tensor_tensor(cmb[:], ev[:], od[:], op=op)
                        nc.sync.dma_start(dram_ap(scr_stat2, 0, [[1, 64]]), cmb[:])
                        nc.sync.dma_start(dst[:], dram_ap(scr_stat2, 0, [[1, 64], [0, 2]]))

                    dT = qp.tile([128, 1], F32)
                    nc.vector.scalar_tensor_tensor(dT[:], mx128[:], mn128[:], epsT[:],
                                                   op0=Alu.subtract, op1=Alu.add)
                    rT = qp.tile([128, 1], F32)
                    nc.vector.reciprocal(rT[:], dT[:])
                    nc.vector.tensor_scalar(r256[:], rT[:], 256.0, None, op0=Alu.mult)

                # ========= phase B + hist, chunk-interleaved =========
                with tc.tile_pool(name="hps", bufs=1, space="PSUM") as hp:
                    hist_ps = [hp.tile([128, 512], F32, name=f"histps{b}", tag=f"h{b}")
                               for b in range(4)]
                    # start=True would clear the whole bank, wiping sibling
                    # chains sharing it -> memset once, accumulate start=False
                    for b in range(4):
                        nc.vector.memset(hist_ps[b][:], 0.0)
                    with tc.tile_pool(name="qtmp", bufs=2) as qt, \
                         tc.tile_pool(name="tps", bufs=4, space="PSUM") as tp, \
                         tc.tile_pool(name="ohp", bufs=2) as ohp, \
                         nc.named_scope("hist"):
                        nq = NHALF // QCH
                        nchunk = PE_COLS // HCH  # 30
                        nthr = [0]

                        def q_chunk(k):
                            sl = slice(k * QCH, (k + 1) * QCH)
                            xq = qt.tile([128, QCH], F32, tag="xq")
                            nc.sync.dma_start(xq[:], x_r[:, sl])
                            t_ = qt.tile([128, QCH], F32, tag="t_")
                            nc.vector.tensor_scalar(t_[:], xq[:], mn128[:], r256[:],
                                                    op0=Alu.subtract, op1=Alu.mult)
                            y_ = qt.tile([128, QCH], F32, tag="y_")
                            nc.vector.tensor_scalar(y_[:], t_[:], MAGIC, MAGIC,
                                                    op0=Alu.add, op1=Alu.subtract)
                            g_ = qt.tile([128, QCH], F32, tag="g_")
                            nc.vector.tensor_tensor(g_[:], y_[:], t_[:], op=Alu.is_gt)
                            nc.vector.tensor_tensor(t_[:], y_[:], g_[:], op=Alu.subtract)
                            nc.vector.tensor_scalar(q_sb[:, sl], t_[:], 255.0, None,
                                                    op0=Alu.min)

                        def thrA_inst(t):
                            nc.scalar.activation(
                                junkA[:], q_sb[:, 0:RA], Act.Sign,
                                bias=biasA[:, t - 1:t], scale=1.0,
                                accum_out=accA[:, t - 1:t])

                        def hist_chunk(ci):
                            # HCH cols; hl = [hi(q) | lo(q)] bf16, then 16
                            # contiguous fp8 one-hot scans over both; fp8
                            # DoubleRow matmuls contract 256 pixels (2 tiles)
                            qTc = ohp.tile([128, HCH], BF16, tag="qTc")
                            for tt in range(TPH):
                                t0 = RA // 128 + ci * TPH + tt
                                pt = tp.tile([128, 128], BF16, tag="pt")
                                nc.tensor.transpose(
                                    pt[:], q_sb[:, t0 * 128:(t0 + 1) * 128], identb[:])
                                nc.vector.tensor_copy(qTc[:, tt * 128:(tt + 1) * 128],
                                                      pt[:])
                            t16 = ohp.tile([128, HCH], F32, tag="t16")
                            # t16 = (q-7.5)/16, exact for integer q
                            nc.scalar.activation(t16[:], qTc[:], Act.Identity,
                                                 bias=b716[:], scale=0.0625)
                            hi_bf = ohp.tile([128, HCH], BF16, tag="hi")
                            nc.vector.tensor_scalar(hi_bf[:], t16[:], MAGIC15,
                                                    MAGIC15, op0=Alu.add,
                                                    op1=Alu.subtract)
                            lo_bf = ohp.tile([128, HCH], BF16, tag="lo")
                            nc.vector.scalar_tensor_tensor(lo_bf[:], hi_bf[:],
                                                           -16.0, qTc[:],
                                                           op0=Alu.mult, op1=Alu.add)
                            # one-hots in DR-canonical block layout:
                            # col (g,tt)*128 + (v,e) [slab] / + (e,l) [moving];
                            # contiguous fp8 writes; input bcast over v/l via
                            # stride-0 dim, compared against vpat tensors
                            hiap = hi_bf[:]
                            loap = lo_bf[:]
                            slab8 = ohp.tile([128, 16 * HCH], FP8, tag="slab8")
                            oneM8 = ohp.tile([128, 16 * HCH], FP8, tag="oneM8")
                            vsv = vpatS[:].rearrange("p (t v e) -> p t v e",
                                                     t=TPH, v=16)
                            vmv = vpatM[:].rearrange("p (t e l) -> p t e l",
                                                     t=TPH, e=8)
                            for g in range(16):
                                so = slab8[:, g * 512:(g + 1) * 512].rearrange(
                                    "p (t v e) -> p t v e", t=TPH, v=16)
                                si = bass.AP(hiap.tensor, hiap.offset + 8 * g,
                                             [list(hiap.ap[0]), [128, TPH],
                                              [0, 16], [1, 8]])
                                nc.vector.tensor_tensor(so, si, vsv,
                                                        op=Alu.is_equal)
                                mo = oneM8[:, g * 512:(g + 1) * 512].rearrange(
                                    "p (t e l) -> p t e l", t=TPH, e=8)
                                mi = bass.AP(loap.tensor, loap.offset + 8 * g,
                                             [list(loap.ap[0]), [128, TPH],
                                              [1, 8], [0, 16]])
                                nc.vector.tensor_tensor(mo, mi, vmv,
                                                        op=Alu.is_equal)
                            for g in range(16):
                                for tp_ in range(TPH // 2):
                                    base = (g * TPH + 2 * tp_) * 128
                                    slab_ap = slab8[:, base:base + 256].rearrange(
                                        "p (two m) -> p two m", two=2)
                                    mov_ap = oneM8[:, base:base + 256].rearrange(
                                        "p (two m) -> p two m", two=2)
                                    nc.tensor.matmul(
                                        hist_ps[g // 4][:, (g % 4) * 128:(g % 4 + 1) * 128],
                                        slab_ap, mov_ap,
                                        start=False,
                                        stop=(ci == nchunk - 1 and tp_ == TPH // 2 - 1),
                                        perf_mode=mybir.MatmulPerfMode.DoubleRow,
                                        skip_group_check=True)

                        for k in range(nq):
                            q_chunk(k)
                            if k < 2:
                                continue
                            for j in range(2):
                                hist_chunk(2 * (k - 2) + j)
                                # spread ACT threshold instrs across chunks
                                take = min(255 - nthr[0], 5)
                                for tj in range(take):
                                    thrA_inst(nthr[0] + 1 + tj)
                                nthr[0] += take
                        for t in range(nthr[0] + 1, 256):
                            thrA_inst(t)

                    # ============ extract PE counts ============
                    stg = qp.tile([128, 2048], F32)
                    with nc.named_scope("hext"):
                        for b in range(4):
                            nc.scalar.copy(stg[:, b * 512:(b + 1) * 512], hist_ps[b][:])
                # psum row i = thi*8+ch, col a*512+b*128+ch*16+tlo
                stgr = stg[:].rearrange("(t e) (a b m) -> t e a b m", e=8, a=4, b=4)
                for ch in range(8):
                    src = stgr[:, ch, :, :, ch * 16:ch * 16 + 16]
                    dst = dram_ap(scr_hpe, (ch % 2) * 16384 + (ch // 2) * 256,
                                  [[16, 16], [4096, 4], [1024, 4], [1, 16]])
                    nc.sync.dma_start(dst, src)
                # thrA route: count_ge[t] = 0.5*sign_sum + RA/2, rows combined
                chalf = qp.tile([128, 1], F32)
                nc.vector.memset(chalf[:], float(RA) / 2.0)
                cge = qp.tile([128, 255], F32)
                nc.vector.scalar_tensor_tensor(cge[:], accA[:], 0.5,
                                               bcast(chalf[:], accA[:]),
                                               op0=Alu.mult, op1=Alu.add)
                nc.sync.dma_start(scr_cge[:], cge[:])
                cgeE = qp.tile([64, 255], F32)
                cgeO = qp.tile([64, 255], F32)
                nc.sync.dma_start(cgeE[:], dram_ap(scr_cge, 0, [[510, 64], [1, 255]]))
                nc.sync.dma_start(cgeO[:], dram_ap(scr_cge, 255, [[510, 64], [1, 255]]))
                cgeC = qp.tile([64, 255], F32)
                nc.vector.tensor_tensor(cgeC[:], cgeE[:], cgeO[:], op=Alu.add)
                bthr = qp.tile([64, NBINS], F32)
                nc.vector.tensor_scalar(bthr[:, 0:1], cgeC[:, 0:1], -1.0,
                                        float(2 * RA), op0=Alu.mult, op1=Alu.add)
                nc.vector.tensor_tensor(bthr[:, 1:255], cgeC[:, 0:254],
                                        cgeC[:, 1:255], op=Alu.subtract)
                nc.vector.tensor_copy(bthr[:, 255:256], cgeC[:, 254:255])

                nT = qp.tile([64, NBINS], F32)
                hpe0 = qp.tile([64, NBINS], F32)
                hpe1 = qp.tile([64, NBINS], F32)
                nc.sync.dma_start(hpe0[:], dram_ap(scr_hpe, 0, [[256, 64], [1, 256]]))
                nc.sync.dma_start(hpe1[:], dram_ap(scr_hpe, 16384, [[256, 64], [1, 256]]))
                nc.vector.tensor_tensor(nT[:], hpe0[:], hpe1[:], op=Alu.add)
                nc.vector.tensor_tensor(nT[:], nT[:], bthr[:], op=Alu.add)
                nc.sync.dma_start(dbg_n[:], nT[:])

                # ============ entropy ============
                with nc.named_scope("phD"):
                    histT = qp.tile([64, NBINS], F32)
                    nc.vector.tensor_scalar(histT[:], nT[:], 1e-8, None, op0=Alu.add)
                    S_ = qp.tile([64, 1], F32)
                    nc.vector.tensor_reduce(S_[:], histT[:], axis=AxX, op=Alu.add)
                    rS = qp.tile([64, 1], F32)
                    nc.vector.reciprocal(rS[:], S_[:])
                    probT = qp.tile([64, NBINS], F32)
                    nc.vector.tensor_scalar(probT[:], histT[:], rS[:], None, op0=Alu.mult)
                    pe_ = qp.tile([64, NBINS], F32)
                    nc.vector.tensor_scalar(pe_[:], probT[:], 1e-8, None, op0=Alu.add)
                    lnT = qp.tile([64, NBINS], F32)
                    nc.scalar.activation(lnT[:], pe_[:], Act.Ln, bias=zb[0:64, :], scale=1.0)
                    termT = qp.tile([64, NBINS], F32)
                    nc.vector.tensor_tensor(termT[:], probT[:], lnT[:], op=Alu.mult)
                    actT = qp.tile([64, 1], F32)
                    nc.vector.tensor_reduce(actT[:], termT[:], axis=AxX, op=Alu.add,
                                            negate=True)
                    nc.sync.dma_start(dbg_act[:], actT[:])

                # ============ top-16 + selection metadata ============
                with nc.named_scope("topk"):
                    nc.sync.dma_start(dram_ap(scr_act, 0, [[1, 64]]), actT[:])
                    act64 = qp.tile([1, 64], F32)
                    nc.sync.dma_start(act64[:], dram_ap(scr_act, 0, [[64, 1], [1, 64]]))
                    idx16 = qp.tile([1, 16], U32)
                    m8 = qp.tile([1, 8], F32)
                    nc.vector.max(m8[:], act64[:])
                    nc.vector.max_index(idx16[:, 0:8], m8[:], act64[:])
                    act64b = qp.tile([1, 64], F32)
                    nc.vector.match_replace(act64b[:], m8[:], act64[:], -3.0e38)
                    m8b = qp.tile([1, 8], F32)
                    nc.vector.max(m8b[:], act64b[:])
                    nc.vector.max_index(idx16[:, 8:16], m8b[:], act64b[:])
                    act64c = qp.tile([1, 64], F32)
                    nc.vector.match_replace(act64c[:], m8b[:], act64b[:], -3.0e38)
                    nc.sync.dma_start(dbg_idx[:], idx16[:])

                    idx16f = qp.tile([1, 16], F32)
                    nc.vector.tensor_copy(idx16f[:], idx16[:])
                    nc.sync.dma_start(dram_ap(scr_idx, 0, [[1, 16]]), idx16f[:])
                    idx16T = qp.tile([16, 1], F32)
                    nc.sync.dma_start(idx16T[:], dram_ap(scr_idx, 0, [[1, 16], [16, 1]]))
                    goffF8 = qp.tile([16, 16], F32)
                    for j in range(16):
                        nc.vector.tensor_scalar(goffF8[:, j:j + 1], idx16T[:], 16.0,
                                                float(j), op0=Alu.mult, op1=Alu.add)
                    nc.vector.tensor_copy(goffI8[:], goffF8[:])

                    mask01 = qp.tile([1, 64], F32)
                    nc.vector.tensor_scalar(mask01[:], act64c[:], -1.0e38, None,
                                            op0=Alu.is_le)
                    rank = qp.tile([1, 64], F32)
                    nc.vector.tensor_tensor_scan(rank[:], mask01[:], z64[:], 0.0,
                                                 op0=Alu.add, op1=Alu.add)
                    sl1 = qp.tile([1, 64], F32)
                    nc.vector.tensor_tensor(sl1[:], colio[:], rank[:], op=Alu.subtract)
                    sl2 = qp.tile([1, 64], F32)
                    nc.vector.tensor_scalar(sl2[:], sl1[:], 64.0, None, op0=Alu.add)
                    slotf = qp.tile([1, 64], F32)
                    nc.vector.scalar_tensor_tensor(slotf[:], mask01[:], 100000.0, sl2[:],
                                                   op0=Alu.mult, op1=Alu.add)
                    nc.sync.dma_start(dram_ap(scr_slot, 0, [[1, 64]]), slotf[:])
                    slotT = qp.tile([64, 1], F32)
                    nc.sync.dma_start(slotT[:], dram_ap(scr_slot, 0, [[1, 64], [64, 1]]))
                    nc.vector.tensor_scalar(slotI32[:], slotT[:], float(C_OUT), None,
                                            op0=Alu.min)

                # ====== unselected passthrough: 64 cond DRAM->DRAM DMAs ======
                # slot c = output channel (64..111) if unselected, ~100064 if
                # selected (clipped) -> cond (slot < C_OUT) skips selected.
                with nc.named_scope("pass"), nc.scalar.register() as sreg:
                    for c in range(C):
                        nc.scalar.load(sreg, slotI32[c:c + 1, 0:1])
                        idx = nc.scalar.snap(sreg, min_val=0, max_val=C_OUT)
                        dst = bass.AP(out_ext, idx * N, [[16384, 4], [1, 16384]])
                        src = x_ext[c].rearrange("(a m) -> a m", m=16384)
                        nc.scalar.dma_start(dst, src, cond=idx <= C_OUT - 1)
            # qpool closed: q_sb etc freed

            # ============ conv weights: [(ky, ic), (kx, oc)] ============
            wsb = pp.tile([48, 3 * OC], BF16)
            with tc.tile_pool(name="wtmp", bufs=1) as wt:
                wsbf = wt.tile([48, 3 * OC], F32)
                nc.sync.dma_start(wsbf[:], w_ext[:])
                nc.vector.tensor_copy(wsb[:], wsbf[:])
            biasC = pp.tile([OC, 1], F32)
            nc.sync.dma_start(biasC[:], bias_ext[:])

            # === selected gather (3 indirect DMAs from bf16 mirror) + conv
            # selb3[(ky, ic), y*PITCH + c] = xpad[ic, y+ky, c]; contraction
            # packs (ky, ic)=48, one matmul per kx, 2 output rows per matmul.
            with tc.tile_pool(name="convp", bufs=1) as cp, nc.named_scope("conv"):
                selb3 = cp.tile([48, SELW3], BF16)
                selv3 = selb3[:].rearrange("p (r w) -> p r w", w=PITCH)
                nc.vector.memset(selv3[:, 0:129, :], 0.0)
                nc.vector.memset(selv3[:, 129:H + 1, :], 0.0)
                # 16 gathers [16ch, 4096] = 16 img rows each (c*16+j slices),
                # then 3 plain strided DMAs place them per ky:
                #   ky=0: y=r+1 | ky=1: y=r | ky=2: y=r-1
                xbf_sl = dram_ap(xbf_d, 0, [[4096, C * 16], [1, 4096]])
                with tc.tile_pool(name="gathp", bufs=4) as gp:
                    for j in range(16):
                        ga = gp.tile([16, 4096], BF16, tag="ga")
                        nc.gpsimd.indirect_dma_start(
                            out=ga[:], out_offset=None, in_=xbf_sl,
                            in_offset=bass.IndirectOffsetOnAxis(
                                ap=goffI8[:, j:j + 1], axis=0))
                        gav = ga[:].rearrange("p (r w) -> p r w", w=W)
                        nc.sync.dma_start(
                            selv3[0:16, 16 * j + 1:16 * j + 17, 1:1 + W], gav)
                        nc.sync.dma_start(
                            selv3[16:32, 16 * j:16 * j + 16, 1:1 + W], gav)
                        if j == 0:
                            nc.sync.dma_start(selv3[32:48, 0:15, 1:1 + W],
                                              gav[:, 1:16, :])
                        else:
                            nc.sync.dma_start(
                                selv3[32:48, 16 * j - 1:16 * j + 15, 1:1 + W],
                                gav)

                with tc.tile_pool(name="psum", bufs=4, space="PSUM") as psp, \
                     tc.tile_pool(name="stage", bufs=2) as stp:
                    RPS = 8
                    for blk in range(32):
                        stage = stp.tile([OC, RPS * W], F32, tag="stage")
                        for pair in range(RPS // 2):
                            y = blk * RPS + 2 * pair
                            ps = psp.tile([OC, 2 * W], F32, tag="ps")
                            for kx in range(3):
                                rhs = selv3[:, y:y + 2, kx:kx + W]
                                nc.tensor.matmul(
                                    ps[:], wsb[:, kx * OC:(kx + 1) * OC],
                                    rhs, start=(kx == 0), stop=(kx == 2))
                            nc.scalar.activation(stage[:, 2 * pair * W:
                                                       (2 * pair + 2) * W],
                                                 ps[:], Act.Identity, bias=biasC[:],
                                                 scale=1.0)
                        nc.sync.dma_start(
                            out_ext[0:OC, blk * RPS * W:(blk + 1) * RPS * W], stage[:])
    nc.compile()
    return nc


_CACHED = {}


def _get_nc():
    if "nc" not in _CACHED:
        _CACHED["nc"] = build()
    return _CACHED["nc"]


def make_inputs_per_core(x, weight, bias):
    x = np.ascontiguousarray(x, dtype=np.float32)
    weight = np.asarray(weight, dtype=np.float32)
    bias = np.asarray(bias, dtype=np.float32)
    # [(ky, ic), (kx, oc)] for the 48-deep contraction conv
    wt = np.ascontiguousarray(np.transpose(weight, (2, 1, 3, 0)).reshape(48, 3 * OC))
    biasT = np.ascontiguousarray(bias.reshape(OC, 1))
    biasA = np.ascontiguousarray(
        np.broadcast_to(-(np.arange(255, dtype=np.float32) + 0.5), (128, 255)))
    ident = np.ascontiguousarray(np.eye(128, dtype=np.float32))
    colio = np.ascontiguousarray(np.arange(64, dtype=np.float32).reshape(1, 64))
    # (t4, v16, e8) -> v ; (t4, e8, l16) -> l
    vpats = np.ascontiguousarray(np.broadcast_to(
        np.tile(np.repeat(np.arange(16, dtype=np.float32), 8), 4), (128, 512)))
    vpatm = np.ascontiguousarray(np.broadcast_to(
        np.tile(np.arange(16, dtype=np.float32), 4 * 8), (128, 512)))
    maps = []
    for b in range(B):
        maps.append({
            "x": np.ascontiguousarray(x[b].reshape(C, N)),
            "w": wt, "bias": biasT, "biasA": biasA, "ident": ident, "colio": colio,
            "vpats": vpats, "vpatm": vpatm,
        })
    return maps


LAST_RESULT = {}


def kernel(x, weight, bias):
    nc = _get_nc()
    maps = make_inputs_per_core(x, weight, bias)
    trace = bool(int(os.environ.get("KERNEL_TRACE", "0")))
    if trace:
        sys.path.insert(0, os.path.dirname(os.path.abspath(__file__)))
        try:
            import profhook
            profhook.install()
        except Exception:
            trace = False
    res = bu.run_bass_kernel_spmd(nc, maps, list(range(8)), trace=trace)
    LAST_RESULT["res"] = res
    out = np.stack([res.results[i]["out"].reshape(C_OUT, H, W) for i in range(B)])
    return out


if __name__ == "__main__":
    import reference as R
    inputs = R.setup_inputs()
    out = kernel(np.asarray(inputs["x"]), np.asarray(inputs["weight"]),
                 np.asarray(inputs["bias"]))
    print("out shape:", out.shape)



# revision 32
# speedup vs baseline: 1.3427x; 1.3427x over previous
"""AdaptivePConv Trainium2 kernel (8 NeuronCores, data-parallel over batch).

Per core (sample b = core index), on-device:
  1. per-channel min/max (streamed), then exact bins
     q = clip(trunc((x-mn)*recip(mx-mn+eps)*256), 0, 255) stored bf16
     (fp32 magic-number round + is_gt fixup -> exact floor; bf16 holds
      integers 0..255 exactly)
  2. 256-bin histogram entirely on the PE: per 512-col chunk transpose q
     to pixel-major, split into nibbles hi=floor(q/16) (exact via
     RN((q-7.5)/16) since q is integer), lo=q-16*hi; the hi one-hot is
     generated directly in matmul-stationary "slab" layout (col =
     (tile,group)*128 + thi*8 + ch, one strided is_equal per thi), the lo
     one-hot in plane layout read via a 2-dim moving AP; block-diagonal
     [128,128] matmuls accumulate per-stream 16x16 outer-product counts
     (8 (c,h)-streams per matmul) into PSUM
  3. entropy per channel (ACT Ln), top-16 channels via max8/match_replace
  4. selected channels -> 3x3 conv: zero-padded pitched selb, 2 output
     rows per matmul (FD=512 moving, 2-dim AP), bias on ACT
  5. unselected channels scattered to the output tail as 2 big rows per
     channel via indirect DMA (few descriptors), before the conv
"""
import os
import sys
import numpy as np

sys.path.insert(0, "/opt/trn_rl_repo")

import concourse.bass as bass
import concourse.bacc as bacc
import concourse.tile as tile
from concourse import mybir
import concourse.bass_utils as bu

F32 = mybir.dt.float32
BF16 = mybir.dt.bfloat16
FP8 = mybir.dt.float8e4
I32 = mybir.dt.int32
U32 = mybir.dt.uint32
Alu = mybir.AluOpType
Act = mybir.ActivationFunctionType
AxX = mybir.AxisListType.X

B, C, H, W = 8, 64, 256, 256
N = H * W                  # 65536 pixels per channel
NHALF = N // 2             # 32768 per (channel, half) partition row
OC, P_SEL = 64, 16
C_OUT = OC + C - P_SEL     # 112
NBINS = 256
MAGIC = float(np.float32(2.0 ** 23))
MAGIC15 = float(np.float32(1.5 * 2.0 ** 23))  # safe for slightly-negative inputs

RA = 2048                  # leading cols counted via ACT Sign thresholds
PE_COLS = NHALF - RA       # 30720 cols through the PE one-hot route
HCH = 512                  # hist chunk (4 transpose tiles, 2 DR pairs)
TPH = HCH // 128
QCH = 1024                 # q-phase chunk (= 1 hist chunk)
MMCH = 4096                # min/max chunk
PITCH = 258                # padded conv row pitch
SELW3 = PITCH * (H + 1)    # selb3: 257 rows x 258 (row 256 = slack)


def bcast(ap_small, ap_big):
    return bass.broadcast_tensor_aps(ap_small, ap_big)[0]


def build():
    nc = bacc.Bacc()
    x_ext = nc.declare_dram_parameter("x", [C, N], F32, isOutput=False)
    biasA_ext = nc.declare_dram_parameter("biasA", [128, 255], F32, isOutput=False)
    w_ext = nc.declare_dram_parameter("w", [48, 3 * OC], F32, isOutput=False)
    bias_ext = nc.declare_dram_parameter("bias", [OC, 1], F32, isOutput=False)
    ident_ext = nc.declare_dram_parameter("ident", [128, 128], F32, isOutput=False)
    iota_ext = nc.declare_dram_parameter("colio", [1, 64], F32, isOutput=False)
    vpats_ext = nc.declare_dram_parameter("vpats", [128, 512], F32, isOutput=False)
    vpatm_ext = nc.declare_dram_parameter("vpatm", [128, 512], F32, isOutput=False)
    out_ext = nc.declare_dram_parameter("out", [C_OUT, N], F32, isOutput=True)
    dbg_act = nc.declare_dram_parameter("dbg_act", [C, 1], F32, isOutput=True)
    dbg_idx = nc.declare_dram_parameter("dbg_idx", [1, 16], U32, isOutput=True)
    dbg_n = nc.declare_dram_parameter("dbg_n", [C, NBINS], F32, isOutput=True)

    # +W slack: the ky=2 gather's last slice reads 256 elems past channel end
    xbf_d = nc.dram_tensor("xbf", [128 * NHALF + W], BF16)
    scr_stat = nc.dram_tensor("scr_stat", [128], F32)
    scr_stat2 = nc.dram_tensor("scr_stat2", [64], F32)
    scr_cge = nc.dram_tensor("scr_cge", [128, 255], F32)
    scr_act = nc.dram_tensor("scr_act", [64], F32)
    scr_idx = nc.dram_tensor("scr_idx", [16], F32)
    scr_slot = nc.dram_tensor("scr_slot", [64], F32)
    scr_hpe = nc.dram_tensor("scr_hpe", [2 * 64 * 256], F32)

    def dram_ap(t, offset, pattern):
        return bass.AP(t, offset, pattern)

    x_r = x_ext[:].rearrange("c (t m) -> (c t) m", t=2)  # [128, NHALF]

    with tile.TileContext(nc) as tc:
        with tc.tile_pool(name="persist", bufs=1) as pp:
            epsT = pp.tile([128, 1], F32)
            nc.vector.memset(epsT[:], 1e-8)
            zb = pp.tile([128, 1], F32)
            nc.vector.memset(zb[:], 0.0)
            z64 = pp.tile([1, 64], F32)
            nc.vector.memset(z64[:], 0.0)
            colio = pp.tile([1, 64], F32)
            nc.sync.dma_start(colio[:], iota_ext[:])
            vpatS = pp.tile([128, 512], BF16)
            vpatM = pp.tile([128, 512], BF16)
            with tc.tile_pool(name="vptmp", bufs=1) as vpt:
                vpf = vpt.tile([128, 512], F32, tag="vpf")
                nc.sync.dma_start(vpf[:], vpats_ext[:])
                nc.vector.tensor_copy(vpatS[:], vpf[:])
                vpf2 = vpt.tile([128, 512], F32, tag="vpf2")
                nc.sync.dma_start(vpf2[:], vpatm_ext[:])
                nc.vector.tensor_copy(vpatM[:], vpf2[:])
            goffI8 = pp.tile([16, 16], I32)  # ch*16 + j (16th-slice units)
            slotI32 = pp.tile([64, 1], I32)

            with tc.tile_pool(name="qpool", bufs=1) as qp:
                q_sb = qp.tile([128, NHALF], BF16)
                mn128 = qp.tile([128, 1], F32)
                r256 = qp.tile([128, 1], F32)
                identb = qp.tile([128, 128], BF16)
                with tc.tile_pool(name="idtmp", bufs=1) as idt:
                    idf = idt.tile([128, 128], F32)
                    nc.sync.dma_start(idf[:], ident_ext[:])
                    nc.vector.tensor_copy(identb[:], idf[:])
                biasA = qp.tile([128, 255], F32)
                nc.sync.dma_start(biasA[:], biasA_ext[:])
                accA = qp.tile([128, 255], F32)
                junkA = qp.tile([128, RA], BF16)
                b716 = qp.tile([128, 1], F32)
                nc.vector.memset(b716[:], -0.46875)

                # ============ phase A: min/max (streamed) ============
                nmm = NHALF // MMCH
                mxP = qp.tile([128, nmm], F32)
                mnP = qp.tile([128, nmm], F32)
                with tc.tile_pool(name="xmm", bufs=2) as xmp, \
                     nc.named_scope("phA"):
                    for k in range(nmm):
                        sl = slice(k * MMCH, (k + 1) * MMCH)
                        xh = xmp.tile([128, MMCH], F32, tag="xh")
                        nc.sync.dma_start(xh[:], x_r[:, sl])
                        nc.vector.tensor_reduce(mxP[:, k:k + 1], xh[:], axis=AxX, op=Alu.max)
                        nc.vector.tensor_reduce(mnP[:, k:k + 1], xh[:], axis=AxX, op=Alu.min)
                        # bf16 mirror of x for the conv gather (DMA can't cast)
                        xb = xmp.tile([128, MMCH], BF16, tag="xb")
                        nc.scalar.copy(xb[:], xh[:])
                        nc.sync.dma_start(
                            dram_ap(xbf_d, k * MMCH, [[NHALF, 128], [1, MMCH]]),
                            xb[:])
                    mx1 = qp.tile([128, 1], F32)
                    mn1 = qp.tile([128, 1], F32)
                    nc.vector.tensor_reduce(mx1[:], mxP[:], axis=AxX, op=Alu.max)
                    nc.vector.tensor_reduce(mn1[:], mnP[:], axis=AxX, op=Alu.min)

                    # combine (c,half) partition pairs via DRAM bounce
                    mx128 = qp.tile([128, 1], F32)
                    for src1, dst, op in ((mx1, mx128, Alu.max), (mn1, mn128, Alu.min)):
                        nc.sync.dma_start(dram_ap(scr_stat, 0, [[1, 128]]), src1[:])
                        ev = qp.tile([64, 1], F32, tag="ev")
                        od = qp.tile([64, 1], F32, tag="od")
                        nc.sync.dma_start(ev[:], dram_ap(scr_stat, 0, [[2, 64], [1, 1]]))
                        nc.sync.dma_start(od[:], dram_ap(scr_stat, 1, [[2, 64], [1, 1]]))
                        cmb = qp.tile([64, 1], F32, tag="cmb")
                        nc.vector.tensor_tensor(cmb[:], ev[:], od[:], op=op)
                        nc.sync.dma_start(dram_ap(scr_stat2, 0, [[1, 64]]), cmb[:])
                        nc.sync.dma_start(dst[:], dram_ap(scr_stat2, 0, [[1, 64], [0, 2]]))

                    dT = qp.tile([128, 1], F32)
                    nc.vector.scalar_tensor_tensor(dT[:], mx128[:], mn128[:], epsT[:],
                                                   op0=Alu.subtract, op1=Alu.add)
                    rT = qp.tile([128, 1], F32)
                    nc.vector.reciprocal(rT[:], dT[:])
                    nc.vector.tensor_scalar(r256[:], rT[:], 256.0, None, op0=Alu.mult)

                # ========= phase B + hist, chunk-interleaved =========
                with tc.tile_pool(name="hps", bufs=1, space="PSUM") as hp:
                    hist_ps = [hp.tile([128, 512], F32, name=f"histps{b}", tag=f"h{b}")
                               for b in range(4)]
                    # start=True would clear the whole bank, wiping sibling
                    # chains sharing it -> memset once, accumulate start=False
                    for b in range(4):
                        nc.vector.memset(hist_ps[b][:], 0.0)
                    with tc.tile_pool(name="qtmp", bufs=2) as qt, \
                         tc.tile_pool(name="tps", bufs=4, space="PSUM") as tp, \
                         tc.tile_pool(name="ohp", bufs=2) as ohp, \
                         nc.named_scope("hist"):
                        nq = NHALF // QCH
                        nchunk = PE_COLS // HCH  # 30
                        nthr = [0]

                        def q_chunk(k):
                            sl = slice(k * QCH, (k + 1) * QCH)
                            xq = qt.tile([128, QCH], F32, tag="xq")
                            nc.sync.dma_start(xq[:], x_r[:, sl])
                            t_ = qt.tile([128, QCH], F32, tag="t_")
                            nc.vector.tensor_scalar(t_[:], xq[:], mn128[:], r256[:],
                                                    op0=Alu.subtract, op1=Alu.mult)
                            y_ = qt.tile([128, QCH], F32, tag="y_")
                            nc.vector.tensor_scalar(y_[:], t_[:], MAGIC, MAGIC,
                                                    op0=Alu.add, op1=Alu.subtract)
                            g_ = qt.tile([128, QCH], F32, tag="g_")
                            nc.vector.tensor_tensor(g_[:], y_[:], t_[:], op=Alu.is_gt)
                            nc.vector.tensor_tensor(t_[:], y_[:], g_[:], op=Alu.subtract)
                            nc.vector.tensor_scalar(q_sb[:, sl], t_[:], 255.0, None,
                                                    op0=Alu.min)

                        def thrA_inst(t):
                            nc.scalar.activation(
                                junkA[:], q_sb[:, 0:RA], Act.Sign,
                                bias=biasA[:, t - 1:t], scale=1.0,
                                accum_out=accA[:, t - 1:t])

                        def hist_chunk(ci):
                            qTc = ohp.tile([128, HCH], BF16, tag="qTc")
                            for tt in range(TPH):
                                t0 = RA // 128 + ci * TPH + tt
                                pt = tp.tile([128, 128], BF16, tag="pt")
                                nc.tensor.transpose(
                                    pt[:], q_sb[:, t0 * 128:(t0 + 1) * 128], identb[:])
                                nc.vector.tensor_copy(qTc[:, tt * 128:(tt + 1) * 128],
                                                      pt[:])
                            t16 = ohp.tile([128, HCH], F32, tag="t16")
                            # t16 = (q-7.5)/16, exact for integer q
                            nc.scalar.activation(t16[:], qTc[:], Act.Identity,
                                                 bias=b716[:], scale=0.0625)
                            hi_bf = ohp.tile([128, HCH], BF16, tag="hi")
                            nc.vector.tensor_scalar(hi_bf[:], t16[:], MAGIC15, MAGIC15,
                                                    op0=Alu.add, op1=Alu.subtract)
                            lo_bf = ohp.tile([128, HCH], BF16, tag="lo")
                            nc.vector.scalar_tensor_tensor(lo_bf[:], hi_bf[:], -16.0,
                                                           qTc[:],
                                                           op0=Alu.mult, op1=Alu.add)
                            # stationary slab: col (tt*16+g)*128 + thi*8 + ch
                            slab = ohp.tile([128, 16 * HCH], BF16, tag="slab")
                            slabv = slab[:].rearrange("p (o v e) -> p v o e", v=16, e=8)
                            for v in range(16):
                                nc.vector.tensor_scalar(slabv[:, v, :, :], hi_bf[:],
                                                        float(v), None, op0=Alu.is_equal)
                            oneL = ohp.tile([128, 16 * HCH], BF16, tag="oneL")
                            for v in range(16):
                                vsl = slice(v * HCH, (v + 1) * HCH)
                                nc.vector.tensor_scalar(oneL[:, vsl], lo_bf[:], float(v),
                                                        None, op0=Alu.is_equal)
                            oneLr = oneL[:].rearrange("p (v m) -> p m v", v=16)
                            for tt in range(TPH):
                                for g in range(16):
                                    o0 = (tt * 16 + g) * 128
                                    c0 = tt * 128 + 8 * g
                                    nc.tensor.matmul(
                                        hist_ps[g // 4][:, (g % 4) * 128:(g % 4 + 1) * 128],
                                        slab[:, o0:o0 + 128],
                                        oneLr[:, c0:c0 + 8, :],
                                        start=False,
                                        stop=(ci == nchunk - 1 and tt == TPH - 1),
                                        skip_group_check=True)

                        for k in range(nq):
                            q_chunk(k)
                            if k < 2:
                                continue
                            for j in range(2):
                                hist_chunk(2 * (k - 2) + j)
                                # spread ACT threshold instrs across chunks
                                take = min(255 - nthr[0], 5)
                                for tj in range(take):
                                    thrA_inst(nthr[0] + 1 + tj)
                                nthr[0] += take
                        for t in range(nthr[0] + 1, 256):
                            thrA_inst(t)

                    # ============ extract PE counts ============
                    stg = qp.tile([128, 2048], F32)
                    with nc.named_scope("hext"):
                        for b in range(4):
                            nc.scalar.copy(stg[:, b * 512:(b + 1) * 512], hist_ps[b][:])
                # psum row i = thi*8+ch, col a*512+b*128+ch*16+tlo
                stgr = stg[:].rearrange("(t e) (a b m) -> t e a b m", e=8, a=4, b=4)
                for ch in range(8):
                    src = stgr[:, ch, :, :, ch * 16:ch * 16 + 16]
                    dst = dram_ap(scr_hpe, (ch % 2) * 16384 + (ch // 2) * 256,
                                  [[16, 16], [4096, 4], [1024, 4], [1, 16]])
                    nc.sync.dma_start(dst, src)
                # thrA route: count_ge[t] = 0.5*sign_sum + RA/2, rows combined
                chalf = qp.tile([128, 1], F32)
                nc.vector.memset(chalf[:], float(RA) / 2.0)
                cge = qp.tile([128, 255], F32)
                nc.vector.scalar_tensor_tensor(cge[:], accA[:], 0.5,
                                               bcast(chalf[:], accA[:]),
                                               op0=Alu.mult, op1=Alu.add)
                nc.sync.dma_start(scr_cge[:], cge[:])
                cgeE = qp.tile([64, 255], F32)
                cgeO = qp.tile([64, 255], F32)
                nc.sync.dma_start(cgeE[:], dram_ap(scr_cge, 0, [[510, 64], [1, 255]]))
                nc.sync.dma_start(cgeO[:], dram_ap(scr_cge, 255, [[510, 64], [1, 255]]))
                cgeC = qp.tile([64, 255], F32)
                nc.vector.tensor_tensor(cgeC[:], cgeE[:], cgeO[:], op=Alu.add)
                bthr = qp.tile([64, NBINS], F32)
                nc.vector.tensor_scalar(bthr[:, 0:1], cgeC[:, 0:1], -1.0,
                                        float(2 * RA), op0=Alu.mult, op1=Alu.add)
                nc.vector.tensor_tensor(bthr[:, 1:255], cgeC[:, 0:254],
                                        cgeC[:, 1:255], op=Alu.subtract)
                nc.vector.tensor_copy(bthr[:, 255:256], cgeC[:, 254:255])

                nT = qp.tile([64, NBINS], F32)
                hpe0 = qp.tile([64, NBINS], F32)
                hpe1 = qp.tile([64, NBINS], F32)
                nc.sync.dma_start(hpe0[:], dram_ap(scr_hpe, 0, [[256, 64], [1, 256]]))
                nc.sync.dma_start(hpe1[:], dram_ap(scr_hpe, 16384, [[256, 64], [1, 256]]))
                nc.vector.tensor_tensor(nT[:], hpe0[:], hpe1[:], op=Alu.add)
                nc.vector.tensor_tensor(nT[:], nT[:], bthr[:], op=Alu.add)
                nc.sync.dma_start(dbg_n[:], nT[:])

                # ============ entropy ============
                with nc.named_scope("phD"):
                    histT = qp.tile([64, NBINS], F32)
                    nc.vector.tensor_scalar(histT[:], nT[:], 1e-8, None, op0=Alu.add)
                    S_ = qp.tile([64, 1], F32)
                    nc.vector.tensor_reduce(S_[:], histT[:], axis=AxX, op=Alu.add)
                    rS = qp.tile([64, 1], F32)
                    nc.vector.reciprocal(rS[:], S_[:])
                    probT = qp.tile([64, NBINS], F32)
                    nc.vector.tensor_scalar(probT[:], histT[:], rS[:], None, op0=Alu.mult)
                    pe_ = qp.tile([64, NBINS], F32)
                    nc.vector.tensor_scalar(pe_[:], probT[:], 1e-8, None, op0=Alu.add)
                    lnT = qp.tile([64, NBINS], F32)
                    nc.scalar.activation(lnT[:], pe_[:], Act.Ln, bias=zb[0:64, :], scale=1.0)
                    termT = qp.tile([64, NBINS], F32)
                    nc.vector.tensor_tensor(termT[:], probT[:], lnT[:], op=Alu.mult)
                    actT = qp.tile([64, 1], F32)
                    nc.vector.tensor_reduce(actT[:], termT[:], axis=AxX, op=Alu.add,
                                            negate=True)
                    nc.sync.dma_start(dbg_act[:], actT[:])

                # ============ top-16 + selection metadata ============
                with nc.named_scope("topk"):
                    nc.sync.dma_start(dram_ap(scr_act, 0, [[1, 64]]), actT[:])
                    act64 = qp.tile([1, 64], F32)
                    nc.sync.dma_start(act64[:], dram_ap(scr_act, 0, [[64, 1], [1, 64]]))
                    idx16 = qp.tile([1, 16], U32)
                    m8 = qp.tile([1, 8], F32)
                    nc.vector.max(m8[:], act64[:])
                    nc.vector.max_index(idx16[:, 0:8], m8[:], act64[:])
                    act64b = qp.tile([1, 64], F32)
                    nc.vector.match_replace(act64b[:], m8[:], act64[:], -3.0e38)
                    m8b = qp.tile([1, 8], F32)
                    nc.vector.max(m8b[:], act64b[:])
                    nc.vector.max_index(idx16[:, 8:16], m8b[:], act64b[:])
                    act64c = qp.tile([1, 64], F32)
                    nc.vector.match_replace(act64c[:], m8b[:], act64b[:], -3.0e38)
                    nc.sync.dma_start(dbg_idx[:], idx16[:])

                    idx16f = qp.tile([1, 16], F32)
                    nc.vector.tensor_copy(idx16f[:], idx16[:])
                    nc.sync.dma_start(dram_ap(scr_idx, 0, [[1, 16]]), idx16f[:])
                    idx16T = qp.tile([16, 1], F32)
                    nc.sync.dma_start(idx16T[:], dram_ap(scr_idx, 0, [[1, 16], [16, 1]]))
                    goffF8 = qp.tile([16, 16], F32)
                    for j in range(16):
                        nc.vector.tensor_scalar(goffF8[:, j:j + 1], idx16T[:], 16.0,
                                                float(j), op0=Alu.mult, op1=Alu.add)
                    nc.vector.tensor_copy(goffI8[:], goffF8[:])

                    mask01 = qp.tile([1, 64], F32)
                    nc.vector.tensor_scalar(mask01[:], act64c[:], -1.0e38, None,
                                            op0=Alu.is_le)
                    rank = qp.tile([1, 64], F32)
                    nc.vector.tensor_tensor_scan(rank[:], mask01[:], z64[:], 0.0,
                                                 op0=Alu.add, op1=Alu.add)
                    sl1 = qp.tile([1, 64], F32)
                    nc.vector.tensor_tensor(sl1[:], colio[:], rank[:], op=Alu.subtract)
                    sl2 = qp.tile([1, 64], F32)
                    nc.vector.tensor_scalar(sl2[:], sl1[:], 64.0, None, op0=Alu.add)
                    slotf = qp.tile([1, 64], F32)
                    nc.vector.scalar_tensor_tensor(slotf[:], mask01[:], 100000.0, sl2[:],
                                                   op0=Alu.mult, op1=Alu.add)
                    nc.sync.dma_start(dram_ap(scr_slot, 0, [[1, 64]]), slotf[:])
                    slotT = qp.tile([64, 1], F32)
                    nc.sync.dma_start(slotT[:], dram_ap(scr_slot, 0, [[1, 64], [64, 1]]))
                    nc.vector.tensor_scalar(slotI32[:], slotT[:], float(C_OUT), None,
                                            op0=Alu.min)

                # ====== unselected passthrough: 64 cond DRAM->DRAM DMAs ======
                # slot c = output channel (64..111) if unselected, ~100064 if
                # selected (clipped) -> cond (slot < C_OUT) skips selected.
                with nc.named_scope("pass"), nc.scalar.register() as sreg:
                    for c in range(C):
                        nc.scalar.load(sreg, slotI32[c:c + 1, 0:1])
                        idx = nc.scalar.snap(sreg, min_val=0, max_val=C_OUT)
                        dst = bass.AP(out_ext, idx * N, [[16384, 4], [1, 16384]])
                        src = x_ext[c].rearrange("(a m) -> a m", m=16384)
                        nc.scalar.dma_start(dst, src, cond=idx <= C_OUT - 1)
            # qpool closed: q_sb etc freed

            # ============ conv weights: [(ky, ic), (kx, oc)] ============
            wsb = pp.tile([48, 3 * OC], BF16)
            with tc.tile_pool(name="wtmp", bufs=1) as wt:
                wsbf = wt.tile([48, 3 * OC], F32)
                nc.sync.dma_start(wsbf[:], w_ext[:])
                nc.vector.tensor_copy(wsb[:], wsbf[:])
            biasC = pp.tile([OC, 1], F32)
            nc.sync.dma_start(biasC[:], bias_ext[:])

            # === selected gather (3 indirect DMAs from bf16 mirror) + conv
            # selb3[(ky, ic), y*PITCH + c] = xpad[ic, y+ky, c]; contraction
            # packs (ky, ic)=48, one matmul per kx, 2 output rows per matmul.
            with tc.tile_pool(name="convp", bufs=1) as cp, nc.named_scope("conv"):
                selb3 = cp.tile([48, SELW3], BF16)
                selv3 = selb3[:].rearrange("p (r w) -> p r w", w=PITCH)
                nc.vector.memset(selv3[:, 0:129, :], 0.0)
                nc.vector.memset(selv3[:, 129:H + 1, :], 0.0)
                # 16 gathers [16ch, 4096] = 16 img rows each (c*16+j slices),
                # then 3 plain strided DMAs place them per ky:
                #   ky=0: y=r+1 | ky=1: y=r | ky=2: y=r-1
                xbf_sl = dram_ap(xbf_d, 0, [[4096, C * 16], [1, 4096]])
                with tc.tile_pool(name="gathp", bufs=4) as gp:
                    for j in range(16):
                        ga = gp.tile([16, 4096], BF16, tag="ga")
                        nc.gpsimd.indirect_dma_start(
                            out=ga[:], out_offset=None, in_=xbf_sl,
                            in_offset=bass.IndirectOffsetOnAxis(
                                ap=goffI8[:, j:j + 1], axis=0))
                        gav = ga[:].rearrange("p (r w) -> p r w", w=W)
                        nc.sync.dma_start(
                            selv3[0:16, 16 * j + 1:16 * j + 17, 1:1 + W], gav)
                        nc.sync.dma_start(
                            selv3[16:32, 16 * j:16 * j + 16, 1:1 + W], gav)
                        if j == 0:
                            nc.sync.dma_start(selv3[32:48, 0:15, 1:1 + W],
                                              gav[:, 1:16, :])
                        else:
                            nc.sync.dma_start(
                                selv3[32:48, 16 * j - 1:16 * j + 15, 1:1 + W],
                                gav)

                with tc.tile_pool(name="psum", bufs=4, space="PSUM") as psp, \
                     tc.tile_pool(name="stage", bufs=2) as stp:
                    RPS = 8
                    for blk in range(32):
                        stage = stp.tile([OC, RPS * W], F32, tag="stage")
                        for pair in range(RPS // 2):
                            y = blk * RPS + 2 * pair
                            ps = psp.tile([OC, 2 * W], F32, tag="ps")
                            for kx in range(3):
                                rhs = selv3[:, y:y + 2, kx:kx + W]
                                nc.tensor.matmul(
                                    ps[:], wsb[:, kx * OC:(kx + 1) * OC],
                                    rhs, start=(kx == 0), stop=(kx == 2))
                            nc.scalar.activation(stage[:, 2 * pair * W:
                                                       (2 * pair + 2) * W],
                                                 ps[:], Act.Identity, bias=biasC[:],
                                                 scale=1.0)
                        nc.sync.dma_start(
                            out_ext[0:OC, blk * RPS * W:(blk + 1) * RPS * W], stage[:])
    nc.compile()
    return nc


_CACHED = {}


def _get_nc():
    if "nc" not in _CACHED:
        _CACHED["nc"] = build()
    return _CACHED["nc"]


def make_inputs_per_core(x, weight, bias):
    x = np.ascontiguousarray(x, dtype=np.float32)
    weight = np.asarray(weight, dtype=np.float32)
    bias = np.asarray(bias, dtype=np.float32)
    # [(ky, ic), (kx, oc)] for the 48-deep contraction conv
    wt = np.ascontiguousarray(np.transpose(weight, (2, 1, 3, 0)).reshape(48, 3 * OC))
    biasT = np.ascontiguousarray(bias.reshape(OC, 1))
    biasA = np.ascontiguousarray(
        np.broadcast_to(-(np.arange(255, dtype=np.float32) + 0.5), (128, 255)))
    ident = np.ascontiguousarray(np.eye(128, dtype=np.float32))
    colio = np.ascontiguousarray(np.arange(64, dtype=np.float32).reshape(1, 64))
    # (t4, v16, e8) -> v ; (t4, e8, l16) -> l
    vpats = np.ascontiguousarray(np.broadcast_to(
        np.tile(np.repeat(np.arange(16, dtype=np.float32), 8), 4), (128, 512)))
    vpatm = np.ascontiguousarray(np.broadcast_to(
        np.tile(np.arange(16, dtype=np.float32), 4 * 8), (128, 512)))
    maps = []
    for b in range(B):
        maps.append({
            "x": np.ascontiguousarray(x[b].reshape(C, N)),
            "w": wt, "bias": biasT, "biasA": biasA, "ident": ident, "colio": colio,
            "vpats": vpats, "vpatm": vpatm,
        })
    return maps


LAST_RESULT = {}


def kernel(x, weight, bias):
    nc = _get_nc()
    maps = make_inputs_per_core(x, weight, bias)
    trace = bool(int(os.environ.get("KERNEL_TRACE", "0")))
    if trace:
        sys.path.insert(0, os.path.dirname(os.path.abspath(__file__)))
        try:
            import profhook
            profhook.install()
        except Exception:
            trace = False
    res = bu.run_bass_kernel_spmd(nc, maps, list(range(8)), trace=trace)
    LAST_RESULT["res"] = res
    out = np.stack([res.results[i]["out"].reshape(C_OUT, H, W) for i in range(B)])
    return out


if __name__ == "__main__":
    import reference as R
    inputs = R.setup_inputs()
    out = kernel(np.asarray(inputs["x"]), np.asarray(inputs["weight"]),
                 np.asarray(inputs["bias"]))
    print("out shape:", out.shape)

